# revision 30
# baseline (speedup 1.0000x reference)
"""Trainium2 Bass kernel for LogicGatedSNN.

Math:
  w = ternarize(synapse_states)                  # {-1,0,1}
  current = spike_input @ w.T
  spikes[b,o] = (DECAY*vmem[o] + current*(1-refr) >= thr[o])

Implementation (byte-packed fp8, W-stationary, transposed output):
  * Weights: w2 = sign(x-1) + sign(x+1) in {-2,0,2}, produced directly
    in fp8e4 (two ACT sign ops + one DVE add).  Thresholds are doubled:
    compare current2 >= T2, T2 = 2*(thr - DECAY*vmem) or +-2e30 for
    refractory neurons.  All values exact in fp8e4; PSUM accumulates
    fp32 -> bit-exact vs the fp32 reference.
  * Spikes are cast fp32 -> fp8e4 during the SWDGE load.  Both operands
    are transposed as PACKED u16 pairs of fp8 through the xbar (halving
    transpose bytes): tile [128, KC16, x] u16 where element [p, m, x]
    holds the fp8 pair k = 256*m + 2p {+0,+1}.  ALL transposes go on
    the single sync HWDGE ring -- concurrent xbar transposes issued
    from both rings corrupt data on HW.
  * Matmul (DoubleRowSwInterleave, contraction 256/instr): stationary =
    weight bytes [128, o(step2), i(step1)] per 128-o block j (the HW
    consumes interleaved columns high-to-low, so PSUM partition rows
    are o-reversed; the host un-reverses), moving = spike bytes
    [128, i, b] with N=512 batch columns.  A (j, g) group of 16 MMs
    accumulates psum[128o, 512b] for weight block j x batch group g.
    Groups are emitted in order of estimated operand readiness so the
    PE starts as soon as W block 0 + spike group 0 are transposed and
    overlaps both feed pipelines.
  * Epilogue: per block j, T2 (block-reversed to match psum row order)
    is broadcast along b via a K=1 matmul into t2T[128, j, 512]; each
    group then needs one DVE tensor_tensor is_ge (PSUM vs t2T) -> fp8.
  * Output is spikes.T [os_, bs]; the host un-reverses each 128-row
    block and transposes during the gather.

Sharding: 8 cores = 2 (batch) x 4 (out_features): per core
  spike [2048, 4096], syn [1024, 4096], out.T [1024, 2048].
"""

import sys

if "/opt/trn_rl_repo" not in sys.path:
    sys.path.insert(0, "/opt/trn_rl_repo")

import numpy as np

B, IN, OUT = 4096, 4096, 4096
GB, GO = 2, 4  # core grid: batch x out_features
DECAY = 0.8
_TENSORS = {}


def build_core_program(nc, tc, bs, os_, in_, instance=0):
    import concourse.mybir as mybir
    from concourse.bass import ts

    FP32 = mybir.dt.float32
    BF16 = mybir.dt.bfloat16
    FP8 = mybir.dt.float8e4
    Op = mybir.AluOpType
    Act = mybir.ActivationFunctionType
    DRS = mybir.MatmulPerfMode.DoubleRowSwInterleave

    spike = nc.dram_tensor("spike", [bs, in_], FP32, kind="ExternalInput")
    syn = nc.dram_tensor("syn", [os_, in_], FP32, kind="ExternalInput")
    thr = nc.dram_tensor("thr", [1, os_], FP32, kind="ExternalInput")
    vmem = nc.dram_tensor("vmem", [1, os_], FP32, kind="ExternalInput")
    refrac = nc.dram_tensor("refrac", [1, os_], FP32, kind="ExternalInput")
    outT = nc.dram_tensor("spikesT", [os_, bs], FP8, kind="ExternalOutput")
    _TENSORS.update(
        spike=spike, syn=syn, thr=thr, vmem=vmem, refrac=refrac, out=outT
    )

    KC16 = in_ // 256  # u16-pair contraction chunks (256 k each)
    NGB = bs // 512  # batch groups (512 b each)
    NJ = os_ // 128  # weight row blocks
    HW = in_ // 2  # half a syn row load

    with (
        tc.tile_pool(name="misc", bufs=1) as misc,
        tc.tile_pool(name="wst", bufs=2) as wst,
        tc.tile_pool(name="wsign", bufs=2) as wsign,
        tc.tile_pool(name="wtern", bufs=1) as wtern,
        tc.tile_pool(name="wf", bufs=1) as wf,
        tc.tile_pool(name="spp", bufs=5) as spp,
        tc.tile_pool(name="sfp", bufs=1) as sfp,
        tc.tile_pool(name="outp", bufs=3) as outp,
        tc.tile_pool(name="psp", bufs=6, space="PSUM") as psp,
        tc.tile_pool(name="psb", bufs=2, space="PSUM") as psb,
    ):
        # ---- threshold tiles (math itself is emitted later as events so
        # the DVE queue is free for the first weight adds)
        a = misc.tile([1, os_], FP32, tag="a")
        b = misc.tile([1, os_], FP32, tag="b")
        r = misc.tile([1, os_], FP32, tag="r")
        d = misc.tile([1, os_], FP32, tag="d")
        nc.scalar.dma_start(b[:], thr[:, :])
        nc.scalar.dma_start(a[:], vmem[:, :])
        nc.scalar.dma_start(r[:], refrac[:, :])
        ones = misc.tile([1, 1], FP32, tag="ones")
        nc.vector.memset(ones[:], 1.0)
        bneg = misc.tile([128, 1], FP32, tag="bneg")
        bpos = misc.tile([128, 1], FP32, tag="bpos")
        nc.vector.memset(bneg[:], -1.0)
        nc.vector.memset(bpos[:], 1.0)
        t2sc = misc.tile([128, NJ], FP32, tag="t2sc")

        def emit_threshold_math():
            nc.vector.tensor_scalar(a[:], a[:], DECAY, None, Op.mult)
            nc.vector.tensor_tensor(a[:], a[:], b[:], Op.subtract)  # decay*v-thr
            nc.vector.tensor_scalar(b[:], a[:], 0.0, None, Op.is_ge)
            nc.vector.tensor_scalar(b[:], b[:], -4e30, 2e30, Op.mult, Op.add)
            nc.vector.tensor_scalar(r[:], r[:], 0.0, None, Op.is_gt)
            # T2 = -2*c0 + r * (big + 2*c0)   -> d
            nc.vector.tensor_scalar(d[:], a[:], 2.0, None, Op.mult)
            nc.vector.tensor_tensor(b[:], b[:], d[:], Op.add)
            nc.vector.tensor_tensor(b[:], b[:], r[:], Op.mult)
            nc.vector.tensor_scalar(d[:], a[:], -2.0, None, Op.mult)
            nc.vector.tensor_tensor(d[:], d[:], b[:], Op.add)  # t2
            # per-block reversed T2 (psum rows within a block o-reversed),
            # then per-partition scaled bias t2sc[:, j] = -1e8 * T2brev
            # (epilogue = ACT Sigmoid(psum*1e8 + t2sc) -> exact 0/1 at
            # saturation; refractory +-2e30*1e8 = +-2e38 finite in fp32)
            t2brev = a  # reuse
            for j in range(NJ):
                src = d[:, j * 128 + 127 : (j * 128 - 1) if j else None : -1]
                nc.vector.tensor_copy(t2brev[:, ts(j, 128)], src)
            for j in range(NJ):
                pb = psb.tile([128, 1], FP32, tag="pb", name="pb")
                nc.tensor.matmul(
                    pb[:], t2brev[:, ts(j, 128)], ones[:], start=True, stop=True
                )
                nc.scalar.activation(
                    t2sc[:, j : j + 1], pb[:], Act.Copy, bias=0.0, scale=-1e8
                )

        # ---- feed pipelines, emitted interleaved by estimated readiness
        # weights: sign(x-1)+sign(x+1) -> fp8 -> packed u16 transpose
        #   Wp[p, m, o] (u16) = w2[o, 256m+2p : 256m+2p+2]
        # spikes: fp32 -> fp8 cast load -> packed u16 transpose into su4[g]
        Wp = wf.tile([128, KC16, os_], BF16, tag="Wp", name="Wp")
        su4 = [
            sfp.tile([128, KC16, 512], BF16, tag=f"su{g}", name=f"su{g}")
            for g in range(NGB)
        ]

        # Queue layout: SWDGE (gpsimd) = spike cast loads ONLY (clean
        # pipeline, ~6us apiece); sync HWDGE ring = syn loads + ALL xbar
        # transposes, emitted in estimated readiness order (FIFO rings
        # head-of-line block otherwise); scalar ring = output stores only.
        # ACT = signs + sigmoid epilogues (no ring DMA blocking it);
        # DVE = threshold math + W adds.
        NBT = bs // 128  # spike tile loads

        # spike feed times (SWDGE serial)
        sp_done = {bt: 7.0 * (bt + 1) for bt in range(NBT)}
        # weight pipeline times (sync-ring st loads paced by sign slots)
        st_done, sign_end = {}, {}
        for j in range(NJ):
            t0 = st_done[j - 1] if j else 0.5
            if j >= 2:
                t0 = max(t0, sign_end[j - 2])
            st_done[j] = t0 + 6.3
            sign_end[j] = st_done[j] + 9.0
        tw_done = {j: st_done[j] + 13.0 for j in range(NJ)}
        tsu_done = {bt: sp_done[bt] + 2.5 for bt in range(NBT)}

        events = []  # (time, seq, fn)
        st_tiles, w2_tiles, sp_tiles = {}, {}, {}

        def ev(time, fn):
            events.append((time, len(events), fn))

        ev(32.0, emit_threshold_math)

        # SWDGE spike loads, in order
        for bt in range(NBT):
            def mksload(bt=bt):
                sp8 = spp.tile([128, in_], FP8, tag="sp8", name="sp8")
                sp_tiles[bt] = sp8
                nc.gpsimd.dma_start(sp8[:], spike[ts(bt, 128), :])  # fp32->fp8
            ev(0.01 * bt, mksload)

        # sync ring: st loads at slot-readiness, transposes at data-readiness
        for j in range(NJ):
            def mkwload(j=j):
                st = wst.tile([128, in_], FP32, tag="st", name="st")
                st_tiles[j] = st
                nc.sync.dma_start(st[:], syn[ts(j, 128), :])
            ev(st_done[j] - 6.3, mkwload)

            def mktern(j=j):
                st = st_tiles[j]
                s1 = wsign.tile([128, in_], FP8, tag="s1", name="s1")
                s2 = wsign.tile([128, in_], FP8, tag="s2", name="s2")
                nc.scalar.activation(s1[:], st[:], Act.Sign, bias=bneg[:])
                nc.scalar.activation(s2[:], st[:], Act.Sign, bias=bpos[:])
                w2 = wtern.tile([128, in_], FP8, tag="w2", name="w2")
                w2_tiles[j] = w2
                nc.vector.tensor_tensor(w2[:], s1[:], s2[:], Op.add)
            ev(st_done[j] + 0.5, mktern)

            def mkwtrans(j=j):
                nc.sync.dma_start_transpose(
                    Wp[:, :, ts(j, 128)], w2_tiles[j][:].bitcast(BF16)
                )
            ev(tw_done[j] - 1.5, mkwtrans)

        for bt in range(NBT):
            def mkstrans(bt=bt):
                g, i = divmod(bt, 4)
                nc.sync.dma_start_transpose(
                    su4[g][:, :, ts(i, 128)], sp_tiles[bt][:].bitcast(BF16)
                )
            ev(sp_done[bt] + 0.5, mkstrans)

        # matmul groups (j, g) by joint readiness; epilogue (ACT sigmoid
        # with per-partition bias) + store emitted as a later event so it
        # cannot head-of-line block feed work on the ACT/scalar queues
        ps_tiles = {}
        for j in range(NJ):
            for g in range(NGB):
                def mkgroup(j=j, g=g):
                    ps = psp.tile([128, 512], FP32, tag="ps", name="ps")
                    ps_tiles[(j, g)] = ps
                    for m in range(KC16):
                        lhsT = (
                            Wp[:, m, ts(j, 128)]
                            .bitcast(FP8)
                            .rearrange("p (o i) -> p o i", i=2)
                        )
                        rhs = (
                            su4[g][:, m, :]
                            .bitcast(FP8)
                            .rearrange("p (b i) -> p i b", i=2)
                        )
                        nc.tensor.matmul(
                            ps[:],
                            lhsT,
                            rhs,
                            start=(m == 0),
                            stop=(m == KC16 - 1),
                            perf_mode=DRS,
                        )

                def mkepi(j=j, g=g):
                    ps = ps_tiles[(j, g)]
                    ob = outp.tile([128, 512], FP8, tag="ob", name="ob")
                    nc.scalar.activation(
                        ob[:], ps[:], Act.Sigmoid,
                        bias=t2sc[:, j : j + 1], scale=1e8,
                    )
                    nc.scalar.dma_start(outT[ts(j, 128), ts(g, 512)], ob[:])

                ready = max(tw_done[j] + 3.0, tsu_done[4 * g + 3] + 1.5)
                ev(ready, mkgroup)
                ev(ready + 8.0, mkepi)

        events.sort(key=lambda e: (e[0], e[1]))
        for _, _, fn in events:
            fn()
    return outT


def make_nc(bs=B // GB, os_=OUT // GO, in_=IN):
    from concourse import bacc
    from concourse.tile import TileContext

    nc = bacc.Bacc(trn_type="TRN2")
    with TileContext(nc) as tc:
        build_core_program(nc, tc, bs, os_, in_)
    nc.compile()
    return nc


_NC_CACHE = {}


def kernel(
    spike_input,
    synapse_states,
    membrane_potential,
    adaptive_threshold,
    refractory_count,
    _return_results=False,
):
    from concourse.bass_utils import run_bass_kernel_spmd

    spike_input = np.ascontiguousarray(np.asarray(spike_input, dtype=np.float32))
    synapse_states = np.ascontiguousarray(np.asarray(synapse_states, dtype=np.float32))
    membrane_potential = np.asarray(membrane_potential, dtype=np.float32)
    adaptive_threshold = np.asarray(adaptive_threshold, dtype=np.float32)
    refractory_count = np.asarray(refractory_count, dtype=np.float32)

    bs, os_ = B // GB, OUT // GO
    if "nc" not in _NC_CACHE:
        _NC_CACHE["nc"] = make_nc(bs, os_, IN)
    nc = _NC_CACHE["nc"]

    in_maps = []
    for c in range(GB * GO):
        bi, oj = divmod(c, GO)
        in_maps.append(
            {
                "spike": spike_input[bi * bs : (bi + 1) * bs],
                "syn": np.ascontiguousarray(synapse_states[oj * os_ : (oj + 1) * os_]),
                "thr": adaptive_threshold[None, oj * os_ : (oj + 1) * os_],
                "vmem": membrane_potential[None, oj * os_ : (oj + 1) * os_],
                "refrac": refractory_count[None, oj * os_ : (oj + 1) * os_],
            }
        )

    res = run_bass_kernel_spmd(nc, in_maps, core_ids=list(range(GB * GO)))

    full = np.empty((B, OUT), dtype=np.float32)
    for c in range(GB * GO):
        bi, oj = divmod(c, GO)
        # outT rows are o-reversed within each 128-row block; un-reverse,
        # then transpose [os_, bs] -> [bs, os_]
        blkT = res.results[c]["spikesT"].astype(np.float32)
        blkT = blkT.reshape(os_ // 128, 128, bs)[:, ::-1, :].reshape(os_, bs)
        full[bi * bs : (bi + 1) * bs, oj * os_ : (oj + 1) * os_] = blkT.T
    if _return_results:
        return full, res
    return full
